# revision 19
# baseline (speedup 1.0000x reference)
"""Trainium2 Bass kernel for the CustomRNN problem (v2).

Model (per batch element b):
    u_t = W_in @ x_t + bias + sigma*sqrt(2*alpha) * noise_t          [N=256]
    r_{t+1} = (1-alpha) * r_t + alpha * relu(W_rec @ r_t + u_t)
    out_t = W_out @ r_{t+1} + b_out                                  [3]

Sharding: data-parallel over batch across 8 cores (32 batch each), weights
replicated.

v2 design notes (vs v1): the per-step serial chain is PE(matmul)->DVE(max)->
PE, whose latency is dominated by fixed costs (PSUM drain, semaphores, DVE
PSUM access).  v2 minimizes per-step chain work:
  - one batch group: a single fused DVE op per step
      H' = max((1-a)*H, S1),  S1 = a*(W_rec r + u) + (1-a)*r   [PSUM]
    (the (1-a) decay rides the W4 diagonals; decay = fp16(0.8) exactly so
    the max identity relu(x)+d = max(x+d, d) stays exact)
  - the drive a*(W_in x + bias) is matmul'd directly into PSUM once per
    8-step block; a*sigma*sqrt(2a)*noise is pre-scaled on host to fp16 and
    injected per block via identity matmuls (no DVE drive work at all)
  - output projection per 8-step block, bias added by the Activation engine
    during the PSUM->SBUF copy (no ones-matmul)
All matmul operands fp16; PSUM accumulation fp32.
"""

import numpy as np

import concourse.bacc as bacc
import concourse.mybir as mybir
from concourse.tile import TileContext, add_dep_helper
from concourse.bass_utils import run_bass_kernel_spmd

ALPHA = 0.2
NOISE_SCALE = 0.05 * float(np.sqrt(2 * ALPHA))
DECAY = float(np.float16(1.0 - ALPHA))   # 0.7998046875, exact in fp16
N = 256
NCORES = 8
BC = 32          # batch per core
F16 = mybir.dt.float16
F32 = mybir.dt.float32

_CACHE = {}


def _build(T, TC, SB, G, reps=1):
    """Build the Bass program: sequence length T, chunk TC, psum block SB,
    G staggered batch-group chains.  A chunk is split into blocks of SB steps
    (one PSUM bank per group-tile; SB*2*(BC/G)*4 must be <= 2048) with a
    ragged final block if SB doesn't divide TC."""
    NCH = T // TC
    GB = BC // G
    assert NCH * TC == T and G * GB == BC and SB * 2 * GB * 4 <= 2048
    BLOCKS = []          # (start_step, n_steps) per block within a chunk
    o = 0
    while o < TC:
        BLOCKS.append((o, min(SB, TC - o)))
        o += SB
    NBLK = len(BLOCKS)
    nc = bacc.Bacc("TRN2", num_devices=NCORES)

    noise_d = nc.dram_tensor("noiset", [128, T, 2 * BC], F16, kind="ExternalInput")
    xta_d = nc.dram_tensor("xta", [4, T, BC], F16, kind="ExternalInput")
    w4_d = nc.dram_tensor("w4", [128, 512], F16, kind="ExternalInput")
    id_d = nc.dram_tensor("ident", [128, 128], F16, kind="ExternalInput")
    win_d = nc.dram_tensor("win", [4, 256], F16, kind="ExternalInput")
    wout_d = nc.dram_tensor("wout", [128, 6], F16, kind="ExternalInput")
    woutb_d = nc.dram_tensor("woutb", [3, 1], F32, kind="ExternalInput")
    y_d = nc.dram_tensor("y", [3, T, BC], F16, kind="ExternalOutput")

    with TileContext(nc) as tc:
        with (
            tc.tile_pool(name="consts", bufs=1) as consts,
            tc.tile_pool(name="hist", bufs=2) as histp,
            tc.tile_pool(name="noise", bufs=2) as noisep,
            tc.tile_pool(name="xtap", bufs=2) as xtap,
            tc.tile_pool(name="ysbp", bufs=2) as ysbp,
            tc.tile_pool(name="pv", bufs=3 * G, space="PSUM") as pvp,
            tc.tile_pool(name="pyp", bufs=2, space="PSUM") as pyp,
        ):
            w4_sb = consts.tile_from(w4_d[:, :])
            id_sb = consts.tile_from(id_d[:, :])
            win_sb = consts.tile_from(win_d[:, :])
            wout_sb = consts.tile_from(wout_d[:, :])
            woutb_sb = consts.tile_from(woutb_d[:, :])

            # Ordering-only (nosync) chain over every PE matmul: pins the
            # scheduler to the emission order.
            _prev_mm = [None]

            def mm(*args, **kw):
                inst = nc.tensor.matmul(*args, **kw)
                raw = getattr(inst, "ins", inst)
                if _prev_mm[0] is not None:
                    add_dep_helper(raw, _prev_mm[0], sync=False,
                                   reason="pe-stream-order")
                _prev_mm[0] = raw
                return inst

            nxt = None              # prefetched (noise_sb, xta_sb) for chunk+1
            for rep in range(reps):
                prev_hist = None
                for ck in range(NCH):
                    ts0 = ck * TC
                    if nxt is None:
                        noise_sb = noisep.tile([128, TC, 2 * BC], F16)
                        nc.sync.dma_start(out=noise_sb[:],
                                          in_=noise_d[:, ts0:ts0 + TC, :])
                        xta_sb = xtap.tile([4, TC, BC], F16)
                        nc.sync.dma_start(out=xta_sb[:],
                                          in_=xta_d[:, ts0:ts0 + TC, :])
                    else:
                        noise_sb, xta_sb = nxt
                    # prefetch next chunk's inputs ahead of this chunk's
                    # y-output DMA in SP program order (SP issues in order and
                    # blocks on each DMA's waits)
                    if ck + 1 < NCH or rep + 1 < reps:
                        nts = (ts0 + TC) % T
                        n2 = noisep.tile([128, TC, 2 * BC], F16, name="noise2")
                        nc.sync.dma_start(out=n2[:],
                                          in_=noise_d[:, nts:nts + TC, :])
                        x2 = xtap.tile([4, TC, BC], F16, name="xta2")
                        nc.sync.dma_start(out=x2[:], in_=xta_d[:, nts:nts + TC, :])
                        nxt = (n2, x2)
                    else:
                        nxt = None
                    noise_r = noise_sb[:].rearrange("p t (c b) -> p t c b", c=2)
                    noise_r2 = (nxt[0][:].rearrange("p t (c b) -> p t c b", c=2)
                                if nxt is not None else None)
                    # hist slot s holds state r_{ts0+s}; slot 0 = carry-in
                    hist = histp.tile([128, TC + 1, 2, BC], F16)
                    ysb = ysbp.tile([3, TC, BC], F16)
                    if ck == 0:
                        nc.vector.memset(hist[:, 0], 0.0)

                    pvs = carry_pvs
                    carry_pvs = {}

                    def emit_drive(key, blk, g, m_c, xt):
                        # PSUM bank protocol: exactly one start=True per bank
                        # (pending-zero is marked at 2KB-bank granularity), so
                        # only each group-tile's FIRST matmul starts; tiles
                        # are <= one bank and bank-aligned by pad_slot_size
                        b0, bn = BLOCKS[blk]
                        gsl = slice(g * GB, (g + 1) * GB)
                        if (key, g) not in pvs:
                            pvs[(key, g)] = pvp.tile([128, bn, 2, GB], F32,
                                                     name="pv", tag="pv")
                        mm(pvs[(key, g)][:, :, m_c, :],
                           win_sb[:, m_c * 128:(m_c + 1) * 128],
                           xt[:, b0:b0 + bn, gsl],
                           start=(m_c == 0), stop=False, skip_group_check=True)

                    def emit_noise(key, blk, g, s0, s1, nr):
                        b0, bn = BLOCKS[blk]
                        gsl = slice(g * GB, (g + 1) * GB)
                        for s in range(s0, min(s1, bn)):
                            mm(pvs[(key, g)][:, s], id_sb[:],
                               nr[:, b0 + s, :, gsl],
                               start=False, stop=False, skip_group_check=True)

                    def emit_y(blk, part):
                        # part 0: k-chunk-0 matmul; part 1: k-chunk-1 + bias
                        b0, bn = BLOCKS[blk]
                        if part == 0:
                            pvs[("y", blk)] = pyp.tile([3, bn, BC], F32,
                                                       name="py", tag="py")
                        py = pvs[("y", blk)]
                        mm(py[:],
                           wout_sb[:, part * 3:(part + 1) * 3],
                           hist[:, 1 + b0:1 + b0 + bn, part, :],
                           start=(part == 0), stop=(part == 1),
                           skip_group_check=True)
                        if part == 1:
                            nc.scalar.activation(
                                ysb[:, b0:b0 + bn, :], py[:],
                                mybir.ActivationFunctionType.Identity,
                                bias=woutb_sb[:])
                            del pvs[("y", blk)]

                    def prefill(blk):
                        for g in range(G):
                            emit_drive(blk, g, 0)
                            emit_drive(blk, g, 1)
                        for g in range(G):
                            emit_noise(blk, g, 0, SB)

                    for blk in range(NBLK):
                        b0, bn = BLOCKS[blk]
                        if blk == 0:
                            # chunk boundary: no previous block to hide in
                            prefill(0)
                        for s in range(bn):
                            l = b0 + s
                            # filler emissions: issued while this step's w4
                            # matmuls wait on the previous state update
                            if s == 1 and blk > 0:
                                emit_y(blk - 1, 0)
                            elif s == 2 and blk > 0:
                                emit_y(blk - 1, 1)
                            elif s == 3 and blk + 1 < NBLK:
                                for g in range(G):
                                    emit_drive(blk + 1, g, 0)
                            elif s == 4 and blk + 1 < NBLK:
                                for g in range(G):
                                    emit_drive(blk + 1, g, 1)
                            elif s == 5 and blk + 1 < NBLK:
                                emit_noise(blk + 1, 0, 0, SB)
                            elif s == 6 and blk + 1 < NBLK and G > 1:
                                emit_noise(blk + 1, 1, 0, SB)
                            if l == 0 and ck > 0:
                                rd, rs = prev_hist, TC
                            else:
                                rd, rs = hist, l
                            for g in range(G):
                                gsl = slice(g * GB, (g + 1) * GB)
                                pv = pvs[(blk, g)]
                                for k_c in range(2):
                                    for m_c in range(2):
                                        mm(pv[:, s, m_c],
                                           w4_sb[:, (2 * k_c + m_c) * 128:
                                                 (2 * k_c + m_c + 1) * 128],
                                           rd[:, rs, k_c, gsl],
                                           start=False, stop=(k_c == 1),
                                           skip_group_check=True)
                                # H' = max((1-a)*H, S1)  (single fused DVE op)
                                nc.vector.scalar_tensor_tensor(
                                    out=hist[:, l + 1, :, gsl],
                                    in0=rd[:, rs, :, gsl],
                                    scalar=DECAY,
                                    in1=pv[:, s],
                                    op0=mybir.AluOpType.mult,
                                    op1=mybir.AluOpType.max)
                        if blk > 0:
                            for g in range(G):
                                del pvs[(blk - 1, g)]
                    emit_y(NBLK - 1, 0)
                    emit_y(NBLK - 1, 1)
                    nc.sync.dma_start(out=y_d[:, ts0:ts0 + TC, :], in_=ysb[:])
                    prev_hist = hist
    nc.finalize()
    return nc


def get_nc(T=1000, TC=100, SB=8, G=2, reps=1):
    key = (T, TC, SB, G, reps)
    if key not in _CACHE:
        _CACHE[key] = _build(T, TC, SB, G, reps)
    return _CACHE[key]


def make_inputs(x, noise, W_in, W_rec, W_out_w, W_out_b, bias):
    """Host-side shard + layout prep.  Returns in_maps for 8 cores.

    Exponential rescaling: the device recurrence uses decay d = fp16(0.8),
    slightly below the true 0.8.  Because relu is positively homogeneous,
    running the recurrence on r~_t = c^t r_t with c = d/0.8 (so 0.8*c = d
    exactly), drive scaled by c^(t+1), and the output rescaled by c^-(t+1)
    on the host reproduces the true-decay dynamics exactly.
    """
    x = np.asarray(x, np.float32)
    noise = np.asarray(noise, np.float32)
    W_in = np.asarray(W_in, np.float32)
    W_rec = np.asarray(W_rec, np.float32)
    W_out_w = np.asarray(W_out_w, np.float32)
    W_out_b = np.asarray(W_out_b, np.float32)
    bias = np.asarray(bias, np.float32)
    B, T, _ = x.shape

    cfac = DECAY / (1.0 - ALPHA)                       # 0.99975586
    tfac = np.power(cfac, np.arange(1, T + 1), dtype=np.float64).astype(np.float32)

    # W4 chunks carry the state decay on their diagonal: W_rec's diagonal is
    # zero, so chunk (k,k)'s diagonal becomes fp16(1-alpha) exactly.
    w4 = np.empty((128, 512), np.float16)
    wrt = ALPHA * cfac * W_rec.T + DECAY * np.eye(256, dtype=np.float32)
    wrt = wrt.astype(np.float16)                       # [k, m]
    for k_c in range(2):
        for m_c in range(2):
            w4[:, (2 * k_c + m_c) * 128:(2 * k_c + m_c + 1) * 128] = \
                wrt[128 * k_c:128 * (k_c + 1), 128 * m_c:128 * (m_c + 1)]
    ident = np.eye(128, dtype=np.float16)
    win = np.empty((4, 256), np.float16)
    win[:3] = (ALPHA * W_in.T).astype(np.float16)
    win[3] = (ALPHA * bias).astype(np.float16)
    wout = np.empty((128, 6), np.float16)
    wt = W_out_w.T.astype(np.float16)              # [n, 3]
    for k_c in range(2):
        wout[:, 3 * k_c:3 * (k_c + 1)] = wt[128 * k_c:128 * (k_c + 1)]
    woutb = np.zeros((3, 1), np.float32)           # bias added on host

    nscale = ALPHA * NOISE_SCALE
    in_maps = []
    for c in range(NCORES):
        b0 = c * BC
        nz = (noise[b0:b0 + BC] * (nscale * tfac[None, :, None])
              ).astype(np.float16)                     # [32, T, 256]
        nzt = np.ascontiguousarray(
            nz.reshape(BC, T, 2, 128).transpose(3, 1, 2, 0)).reshape(128, T, 2 * BC)
        xc = x[b0:b0 + BC] * tfac[None, :, None]       # [32, T, 3]
        xta = np.empty((4, T, BC), np.float16)
        xta[:3] = xc.transpose(2, 1, 0).astype(np.float16)
        xta[3] = tfac[:, None]
        in_maps.append({
            "noiset": nzt, "xta": xta, "w4": w4, "ident": ident,
            "win": win, "wout": wout, "woutb": woutb,
        })
    return in_maps


def gather_output(results, B, T, W_out_b):
    cfac = DECAY / (1.0 - ALPHA)
    inv = np.power(cfac, -np.arange(1, T + 1), dtype=np.float64).astype(np.float32)
    out = np.empty((B, T, 3), np.float32)
    for c in range(NCORES):
        out[c * BC:(c + 1) * BC] = results[c]["y"].transpose(2, 1, 0).astype(np.float32)
    out *= inv[None, :, None]
    out += np.asarray(W_out_b, np.float32)[None, None, :]
    return out


def kernel(x, noise, W_in, W_rec, W_out_w, W_out_b, bias):
    x = np.asarray(x, np.float32)
    B, T, _ = x.shape
    nc = get_nc(T=T)
    in_maps = make_inputs(x, noise, W_in, W_rec, W_out_w, W_out_b, bias)
    res = run_bass_kernel_spmd(nc, in_maps, list(range(NCORES)))
    return gather_output(res.results, B, T, W_out_b)


# revision 38
# speedup vs baseline: 2.9559x; 2.9559x over previous
"""Trainium2 Bass kernel for the CustomRNN problem (v2).

Model (per batch element b):
    u_t = W_in @ x_t + bias + sigma*sqrt(2*alpha) * noise_t          [N=256]
    r_{t+1} = (1-alpha) * r_t + alpha * relu(W_rec @ r_t + u_t)
    out_t = W_out @ r_{t+1} + b_out                                  [3]

Sharding: data-parallel over batch across 8 cores (32 batch each), weights
replicated.

v2 design notes (vs v1): the per-step serial chain is PE(matmul)->DVE(max)->
PE, whose latency is dominated by fixed costs (PSUM drain ~173ns, DVE PSUM
access ~250ns round trip, semaphore hops).  v2 minimizes per-step chain work:
  - G=2 staggered 16-batch group chains, each with a single fused DVE op
    per step:  H' = max((1-a)*H, S1),  S1 = a*(W_rec r + u) + (1-a)*r [PSUM]
    (the (1-a) decay rides the W4 diagonals as fp16(0.8); an exponential
    host-side rescaling r~_t = c^t r_t with c = fp16(0.8)/0.8 makes that
    decay exact, see make_inputs)
  - the drive a*(W_in x + bias) is matmul'd directly into PSUM once per
    8-step block; noise is pre-scaled on host to fp8 (x16, identity diag
    1/16) and injected via identity matmuls (no DVE drive work at all)
  - output projection per 8-step block on PE gaps; output bias on host
  - all block/chunk boundary work (drive, noise, y-proj, DMAs) is
    software-pipelined into the ~370ns/step windows where the next step's
    matmuls wait on the state update, incl. across chunk boundaries
Steady state ~572ns/step (TimelineSim); recurrence matmuls fp16, PSUM fp32.
"""

import numpy as np

import concourse.bacc as bacc
import concourse.mybir as mybir
from concourse.tile import TileContext, add_dep_helper
from concourse.bass_utils import run_bass_kernel_spmd

ALPHA = 0.2
NOISE_SCALE = 0.05 * float(np.sqrt(2 * ALPHA))
DECAY = float(np.float16(1.0 - ALPHA))   # 0.7998046875, exact in fp16
N = 256
NCORES = 8
BC = 32          # batch per core
F16 = mybir.dt.float16
F32 = mybir.dt.float32
F8 = mybir.dt.float8e4      # e4m3
F8NP = mybir.dt.np(mybir.dt.float8e4)
NOISE_PREMUL = 16.0         # fp8 noise stored x16; identity diag = 1/16

_CACHE = {}


def _build(T, TC, SB, G, reps=1):
    """Build the Bass program: sequence length T, max chunk TC, psum block SB,
    G staggered batch-group chains.  Chunks are multiples of SB (a ragged
    final chunk absorbs the remainder) so every block is exactly SB steps
    (one PSUM bank per group-tile; SB*2*(BC/G)*4 must be <= 2048)."""
    GB = BC // G
    assert G * GB == BC and SB * 2 * GB * 4 <= 2048 and TC % SB == 0
    CHUNKS = []          # (start_step, length) per chunk
    o = 0
    while o < T:
        n = min(TC, T - o)
        assert n % SB == 0, f"T={T} not a multiple of SB={SB}"
        CHUNKS.append((o, n))
        o += n
    nc = bacc.Bacc("TRN2", num_devices=NCORES)

    noise_d = nc.dram_tensor("noiset", [128, T, 2 * BC], F8, kind="ExternalInput")
    xta_d = nc.dram_tensor("xta", [4, T, BC], F16, kind="ExternalInput")
    # all fp16 constants in one tensor (one DMA): w4 | win (4 rows) | wout
    # | block-0 xta (4 rows x SB*BC) so the cold start needs just two DMAs
    cpk_d = nc.dram_tensor("cpack", [128, 774 + SB * BC], F16,
                           kind="ExternalInput")
    # fp8: identity/16 | block-0 noise (SB*2*BC cols)
    id_d = nc.dram_tensor("ident", [128, 128 + SB * 2 * BC], F8,
                          kind="ExternalInput")
    y_d = nc.dram_tensor("y", [3, T, BC], F16, kind="ExternalOutput")

    with TileContext(nc) as tc:
        with (
            tc.tile_pool(name="consts", bufs=1) as consts,
            tc.tile_pool(name="hist", bufs=2) as histp,
            tc.tile_pool(name="noise", bufs=2) as noisep,
            tc.tile_pool(name="xtap", bufs=2) as xtap,
            tc.tile_pool(name="ysbp", bufs=2) as ysbp,
            tc.tile_pool(name="pv", bufs=3 * G, space="PSUM") as pvp,
            tc.tile_pool(name="pyp", bufs=2, space="PSUM") as pyp,
        ):
            cpk_sb = consts.tile_from(cpk_d[:, :])
            idp_sb = consts.tile_from(id_d[:, :])
            id_sb = idp_sb[:, 0:128]
            w4_sb = cpk_sb[:, 0:512]
            win_sb = cpk_sb[:, 512:768]
            wout_sb = cpk_sb[:, 768:774]
            # block-0 drive/noise sources packed with the constants
            xta0_sb = cpk_sb[0:4, 774:774 + SB * BC].rearrange(
                "p (t b) -> p t b", t=SB)
            noise0_sb = idp_sb[:, 128:128 + SB * 2 * BC].rearrange(
                "p (t c b) -> p t c b", t=SB, c=2)

            # Ordering-only (nosync) chain over every PE matmul: pins the
            # scheduler to the emission order.
            _prev_mm = [None]

            def mm(*args, **kw):
                inst = nc.tensor.matmul(*args, **kw)
                raw = getattr(inst, "ins", inst)
                if _prev_mm[0] is not None:
                    add_dep_helper(raw, _prev_mm[0], sync=False,
                                   reason="pe-stream-order")
                _prev_mm[0] = raw
                return inst

            nxt = None              # prefetched (noise_sb, xta_sb) for chunk+1
            carry_pvs = {}          # cross-chunk prefilled psum tiles
            carry_y = None          # previous chunk's deferred last y-block
            prev_TC = None
            for rep in range(reps):
                prev_hist = None
                for ck, (ts0, TCk) in enumerate(CHUNKS):
                    NBLK = TCk // SB
                    BLOCKS = [(b * SB, SB) for b in range(NBLK)]
                    if nxt is None:
                        # cold start: block 0's inputs ride the const DMAs,
                        # so only steps SB.. wait on these chunk DMAs
                        noise_sb = noisep.tile([128, TCk, 2 * BC], F8)
                        xta_sb = xtap.tile([4, TCk, BC], F16)
                        nc.sync.dma_start(out=xta_sb[:],
                                          in_=xta_d[:, ts0:ts0 + TCk, :])
                        nc.sync.dma_start(out=noise_sb[:],
                                          in_=noise_d[:, ts0:ts0 + TCk, :])
                    else:
                        noise_sb, xta_sb = nxt
                    # prefetch next chunk's inputs ahead of this chunk's
                    # y-output DMA in SP program order (SP issues in order and
                    # blocks on each DMA's waits)
                    if ck + 1 < len(CHUNKS) or rep + 1 < reps:
                        nts0, nTC = CHUNKS[(ck + 1) % len(CHUNKS)]
                        n2 = noisep.tile([128, nTC, 2 * BC], F8, name="noise2")
                        nc.sync.dma_start(out=n2[:],
                                          in_=noise_d[:, nts0:nts0 + nTC, :])
                        x2 = xtap.tile([4, nTC, BC], F16, name="xta2")
                        nc.sync.dma_start(out=x2[:],
                                          in_=xta_d[:, nts0:nts0 + nTC, :])
                        nxt = (n2, x2)
                    else:
                        nxt = None
                    noise_r = noise_sb[:].rearrange("p t (c b) -> p t c b", c=2)
                    noise_r2 = (nxt[0][:].rearrange("p t (c b) -> p t c b", c=2)
                                if nxt is not None else None)
                    # hist slot s holds state r_{ts0+s}; slot 0 = carry-in
                    hist = histp.tile([128, TCk + 1, 2, BC], F16)
                    ysb = ysbp.tile([3, TCk, BC], F16)
                    if ck == 0:
                        nc.vector.memset(hist[:, 0], 0.0)

                    pvs = carry_pvs
                    carry_pvs = {}

                    def emit_drive(key, blk, g, m_c, xt):
                        # PSUM bank protocol: exactly one start=True per bank
                        # (pending-zero is marked at 2KB-bank granularity), so
                        # only each group-tile's FIRST matmul starts; tiles
                        # are <= one bank and bank-aligned by pad_slot_size
                        b0, bn = BLOCKS[blk]
                        gsl = slice(g * GB, (g + 1) * GB)
                        if (key, g) not in pvs:
                            pvs[(key, g)] = pvp.tile([128, bn, 2, GB], F32,
                                                     name="pv", tag="pv")
                        mm(pvs[(key, g)][:, :, m_c, :],
                           win_sb[0:4, m_c * 128:(m_c + 1) * 128],
                           xt[:, b0:b0 + bn, gsl],
                           start=(m_c == 0), stop=False, skip_group_check=True)

                    def emit_noise(key, blk, g, s0, s1, nr):
                        b0, bn = BLOCKS[blk]
                        gsl = slice(g * GB, (g + 1) * GB)
                        for s in range(s0, min(s1, bn)):
                            mm(pvs[(key, g)][:, s], id_sb[:],
                               nr[:, b0 + s, :, gsl],
                               start=False, stop=False, skip_group_check=True)

                    def emit_y_at(key, b0, bn, hist_, ysb_, part):
                        # part 0: k-chunk-0 matmul; part 1: k-chunk-1 + copy
                        if part == 0:
                            pvs[key] = pyp.tile([3, bn, BC], F32,
                                                name="py", tag="py")
                        py = pvs[key]
                        mm(py[:],
                           wout_sb[:, part * 3:(part + 1) * 3],
                           hist_[:, 1 + b0:1 + b0 + bn, part, :],
                           start=(part == 0), stop=(part == 1),
                           skip_group_check=True)
                        if part == 1:
                            nc.scalar.copy(ysb_[:, b0:b0 + bn, :], py[:])
                            del pvs[key]

                    def emit_y(blk, part):
                        b0, bn = BLOCKS[blk]
                        emit_y_at(("y", blk), b0, bn, hist, ysb, part)

                    for blk in range(NBLK):
                        b0, bn = BLOCKS[blk]
                        if blk == 0 and (0, 0) not in pvs:
                            # very first chunk (or no prefetch): prefill from
                            # the const-packed block-0 copies (already local)
                            for g in range(G):
                                emit_drive(0, 0, g, 0, xta0_sb)
                                emit_drive(0, 0, g, 1, xta0_sb)
                            for g in range(G):
                                emit_noise(0, 0, g, 0, SB, noise0_sb)
                        # next emission target: block blk+1, or the NEXT
                        # chunk's block 0 (using the prefetched tiles)
                        if blk + 1 < NBLK:
                            nkey, nblk, nxta, nnr = blk + 1, blk + 1, xta_sb, noise_r
                        elif noise_r2 is not None:
                            nkey, nblk, nxta, nnr = "n0", 0, nxt[1], noise_r2
                        else:
                            nkey = None
                        for s in range(bn):
                            l = b0 + s
                            # filler emissions: issued while this step's w4
                            # matmuls wait on the previous state update
                            if s == 1 and blk > 0:
                                emit_y(blk - 1, 0)
                            elif s == 2 and blk > 0:
                                emit_y(blk - 1, 1)
                            elif s == 1 and blk == 0 and carry_y is not None:
                                emit_y_at(("ycarry",), *carry_y[:4], 0)
                            elif s == 2 and blk == 0 and carry_y is not None:
                                emit_y_at(("ycarry",), *carry_y[:4], 1)
                                pt0, pTC = carry_y[4:]
                                nc.sync.dma_start(
                                    out=y_d[:, pt0:pt0 + pTC, :],
                                    in_=carry_y[3][:])
                                carry_y = None
                            elif s == 3 and nkey is not None:
                                for g in range(G):
                                    emit_drive(nkey, nblk, g, 0, nxta)
                            elif s == 4 and nkey is not None:
                                for g in range(G):
                                    emit_drive(nkey, nblk, g, 1, nxta)
                            elif s == 5 and nkey is not None:
                                emit_noise(nkey, nblk, 0, 0, SB, nnr)
                            elif s == 6 and nkey is not None and G > 1:
                                emit_noise(nkey, nblk, 1, 0, SB, nnr)
                            if l == 0 and ck > 0:
                                rd, rs = prev_hist, prev_TC
                            else:
                                rd, rs = hist, l
                            for g in range(G):
                                gsl = slice(g * GB, (g + 1) * GB)
                                pv = pvs[(blk, g)]
                                for k_c in range(2):
                                    for m_c in range(2):
                                        mm(pv[:, s, m_c],
                                           w4_sb[:, (2 * k_c + m_c) * 128:
                                                 (2 * k_c + m_c + 1) * 128],
                                           rd[:, rs, k_c, gsl],
                                           start=False, stop=(k_c == 1),
                                           skip_group_check=True)
                                # H' = max((1-a)*H, S1)  (single fused DVE op)
                                nc.vector.scalar_tensor_tensor(
                                    out=hist[:, l + 1, :, gsl],
                                    in0=rd[:, rs, :, gsl],
                                    scalar=DECAY,
                                    in1=pv[:, s],
                                    op0=mybir.AluOpType.mult,
                                    op1=mybir.AluOpType.max)
                        if blk > 0:
                            for g in range(G):
                                del pvs[(blk - 1, g)]
                    if nxt is not None:
                        # defer this chunk's last y-block AND its output DMA
                        # into the next chunk's filler slots (off the chain)
                        carry_y = (BLOCKS[NBLK - 1][0], SB, hist, ysb,
                                   ts0, TCk)
                    else:
                        # final chunk: ship all but the last block's output
                        # early, leaving only a tiny DMA after the last y
                        lb0 = BLOCKS[NBLK - 1][0]
                        nc.sync.dma_start(out=y_d[:, ts0:ts0 + lb0, :],
                                          in_=ysb[:, 0:lb0])
                        emit_y(NBLK - 1, 0)
                        emit_y(NBLK - 1, 1)
                        nc.sync.dma_start(out=y_d[:, ts0 + lb0:ts0 + TCk, :],
                                          in_=ysb[:, lb0:])
                    for g in range(G):
                        if ("n0", g) in pvs:
                            carry_pvs[(0, g)] = pvs.pop(("n0", g))
                    prev_hist, prev_TC = hist, TCk
    nc.finalize()
    return nc


def get_nc(T=1000, TC=96, SB=8, G=2, reps=1):
    key = (T, TC, SB, G, reps)
    if key not in _CACHE:
        _CACHE[key] = _build(T, TC, SB, G, reps)
    return _CACHE[key]


def make_inputs(x, noise, W_in, W_rec, W_out_w, W_out_b, bias):
    """Host-side shard + layout prep.  Returns in_maps for 8 cores.

    Exponential rescaling: the device recurrence uses decay d = fp16(0.8),
    slightly below the true 0.8.  Because relu is positively homogeneous,
    running the recurrence on r~_t = c^t r_t with c = d/0.8 (so 0.8*c = d
    exactly), drive scaled by c^(t+1), and the output rescaled by c^-(t+1)
    on the host reproduces the true-decay dynamics exactly.
    """
    x = np.asarray(x, np.float32)
    noise = np.asarray(noise, np.float32)
    W_in = np.asarray(W_in, np.float32)
    W_rec = np.asarray(W_rec, np.float32)
    W_out_w = np.asarray(W_out_w, np.float32)
    W_out_b = np.asarray(W_out_b, np.float32)
    bias = np.asarray(bias, np.float32)
    B, T, _ = x.shape

    cfac = DECAY / (1.0 - ALPHA)                       # 0.99975586
    tfac = np.power(cfac, np.arange(1, T + 1), dtype=np.float64).astype(np.float32)

    # W4 chunks carry the state decay on their diagonal: W_rec's diagonal is
    # zero, so chunk (k,k)'s diagonal becomes fp16(1-alpha) exactly.
    cpack = np.zeros((128, 774 + 8 * BC), np.float16)  # w4|win|wout|xta blk0
    wrt = ALPHA * cfac * W_rec.T + DECAY * np.eye(256, dtype=np.float32)
    wrt = wrt.astype(np.float16)                       # [k, m]
    for k_c in range(2):
        for m_c in range(2):
            cpack[:, (2 * k_c + m_c) * 128:(2 * k_c + m_c + 1) * 128] = \
                wrt[128 * k_c:128 * (k_c + 1), 128 * m_c:128 * (m_c + 1)]
    ident = np.zeros((128, 128 + 8 * 2 * BC), F8NP)    # I/16 | noise blk0
    ident[:, 0:128] = (np.eye(128, dtype=np.float32) / NOISE_PREMUL).astype(F8NP)
    cpack[:3, 512:768] = (ALPHA * W_in.T).astype(np.float16)
    cpack[3, 512:768] = (ALPHA * bias).astype(np.float16)
    wt = W_out_w.T.astype(np.float16)              # [n, 3]
    for k_c in range(2):
        cpack[:, 768 + 3 * k_c:768 + 3 * (k_c + 1)] = \
            wt[128 * k_c:128 * (k_c + 1)]

    nscale = ALPHA * NOISE_SCALE
    in_maps = []
    for c in range(NCORES):
        b0 = c * BC
        nz = (noise[b0:b0 + BC] * (NOISE_PREMUL * nscale * tfac[None, :, None])
              ).astype(F8NP)                           # [32, T, 256]
        nzt = np.ascontiguousarray(
            nz.reshape(BC, T, 2, 128).transpose(3, 1, 2, 0)).reshape(128, T, 2 * BC)
        xc = x[b0:b0 + BC] * tfac[None, :, None]       # [32, T, 3]
        xta = np.empty((4, T, BC), np.float16)
        xta[:3] = xc.transpose(2, 1, 0).astype(np.float16)
        xta[3] = tfac[:, None]
        cpk = cpack.copy()
        cpk[0:4, 774:774 + 8 * BC] = xta[:, 0:8, :].reshape(4, 8 * BC)
        idp = ident.copy()
        idp[:, 128:128 + 8 * 2 * BC] = nzt[:, 0:8, :].reshape(128, 8 * 2 * BC)
        in_maps.append({
            "noiset": nzt, "xta": xta, "cpack": cpk, "ident": idp,
        })
    return in_maps


def gather_output(results, B, T, W_out_b):
    cfac = DECAY / (1.0 - ALPHA)
    inv = np.power(cfac, -np.arange(1, T + 1), dtype=np.float64).astype(np.float32)
    out = np.empty((B, T, 3), np.float32)
    for c in range(NCORES):
        out[c * BC:(c + 1) * BC] = results[c]["y"].transpose(2, 1, 0).astype(np.float32)
    out *= inv[None, :, None]
    out += np.asarray(W_out_b, np.float32)[None, None, :]
    return out


def kernel(x, noise, W_in, W_rec, W_out_w, W_out_b, bias):
    x = np.asarray(x, np.float32)
    B, T, _ = x.shape
    nc = get_nc(T=T)
    in_maps = make_inputs(x, noise, W_in, W_rec, W_out_w, W_out_b, bias)
    res = run_bass_kernel_spmd(nc, in_maps, list(range(NCORES)))
    return gather_output(res.results, B, T, W_out_b)


# revision 40
# speedup vs baseline: 18.6059x; 6.2944x over previous
"""Trainium2 Bass kernel for the CustomRNN problem (v2).

Model (per batch element b):
    u_t = W_in @ x_t + bias + sigma*sqrt(2*alpha) * noise_t          [N=256]
    r_{t+1} = (1-alpha) * r_t + alpha * relu(W_rec @ r_t + u_t)
    out_t = W_out @ r_{t+1} + b_out                                  [3]

Sharding: data-parallel over batch across 8 cores (32 batch each), weights
replicated.

v2 design notes (vs v1): the per-step serial chain is PE(matmul)->DVE(max)->
PE, whose latency is dominated by fixed costs (PSUM drain ~173ns, DVE PSUM
access ~250ns round trip, semaphore hops).  v2 minimizes per-step chain work:
  - G=2 staggered 16-batch group chains, each with a single fused DVE op
    per step:  H' = max((1-a)*H, S1),  S1 = a*(W_rec r + u) + (1-a)*r [PSUM]
    (the (1-a) decay rides the W4 diagonals as fp16(0.8); an exponential
    host-side rescaling r~_t = c^t r_t with c = fp16(0.8)/0.8 makes that
    decay exact, see make_inputs)
  - the drive a*(W_in x + bias) is matmul'd directly into PSUM once per
    8-step block; noise is pre-scaled on host to fp8 (x16, identity diag
    1/16) and injected via identity matmuls (no DVE drive work at all)
  - output projection per 8-step block on PE gaps; output bias on host
  - all block/chunk boundary work (drive, noise, y-proj, DMAs) is
    software-pipelined into the ~370ns/step windows where the next step's
    matmuls wait on the state update, incl. across chunk boundaries
Steady state ~572ns/step (TimelineSim); recurrence matmuls fp16, PSUM fp32.
"""

import numpy as np

import concourse.bacc as bacc
import concourse.mybir as mybir
from concourse.tile import TileContext, add_dep_helper
from concourse.bass_utils import run_bass_kernel_spmd

ALPHA = 0.2
NOISE_SCALE = 0.05 * float(np.sqrt(2 * ALPHA))
DECAY = float(np.float16(1.0 - ALPHA))   # 0.7998046875, exact in fp16
N = 256
NCORES = 8
BC = 32          # batch per core
F16 = mybir.dt.float16
F32 = mybir.dt.float32
F8 = mybir.dt.float8e4      # e4m3
F8NP = mybir.dt.np(mybir.dt.float8e4)
NOISE_PREMUL = 16.0         # fp8 noise stored x16; identity diag = 1/16

_CACHE = {}


def _build(T, TC, SB, G, reps=1):
    """Build the Bass program: sequence length T, max chunk TC, psum block SB,
    G staggered batch-group chains.  Chunks are multiples of SB (a ragged
    final chunk absorbs the remainder) so every block is exactly SB steps
    (one PSUM bank per group-tile; SB*2*(BC/G)*4 must be <= 2048)."""
    GB = BC // G
    assert G * GB == BC and SB * 2 * GB * 4 <= 2048 and TC % SB == 0
    CHUNKS = []          # (start_step, length) per chunk
    o = 0
    while o < T:
        n = min(TC, T - o)
        assert n % SB == 0, f"T={T} not a multiple of SB={SB}"
        CHUNKS.append((o, n))
        o += n
    nc = bacc.Bacc("TRN2", num_devices=NCORES)

    noise_d = nc.dram_tensor("noiset", [128, T, 2 * BC], F8, kind="ExternalInput")
    xta_d = nc.dram_tensor("xta", [4, T, BC], F16, kind="ExternalInput")
    # all fp16 constants in one tensor (one DMA): w4 | win (4 rows) | wout
    # | block-0 xta (4 rows x SB*BC) so the cold start needs just two DMAs
    cpk_d = nc.dram_tensor("cpack", [128, 774 + SB * BC], F16,
                           kind="ExternalInput")
    # fp8: identity/16 | block-0 noise (SB*2*BC cols)
    id_d = nc.dram_tensor("ident", [128, 128 + SB * 2 * BC], F8,
                          kind="ExternalInput")
    y_d = nc.dram_tensor("y", [3, T, BC], F16, kind="ExternalOutput")

    with TileContext(nc) as tc:
        with (
            tc.tile_pool(name="consts", bufs=1) as consts,
            tc.tile_pool(name="hist", bufs=2) as histp,
            tc.tile_pool(name="noise", bufs=2) as noisep,
            tc.tile_pool(name="xtap", bufs=2) as xtap,
            tc.tile_pool(name="ysbp", bufs=2) as ysbp,
            tc.tile_pool(name="pv", bufs=3 * G, space="PSUM") as pvp,
            tc.tile_pool(name="pyp", bufs=2, space="PSUM") as pyp,
        ):
            cpk_sb = consts.tile_from(cpk_d[:, :])
            idp_sb = consts.tile_from(id_d[:, :])
            id_sb = idp_sb[:, 0:128]
            w4_sb = cpk_sb[:, 0:512]
            win_sb = cpk_sb[:, 512:768]
            wout_sb = cpk_sb[:, 768:774]
            # block-0 drive/noise sources packed with the constants
            xta0_sb = cpk_sb[0:4, 774:774 + SB * BC].rearrange(
                "p (t b) -> p t b", t=SB)
            noise0_sb = idp_sb[:, 128:128 + SB * 2 * BC].rearrange(
                "p (t c b) -> p t c b", t=SB, c=2)

            # Ordering-only (nosync) chain over every PE matmul: pins the
            # scheduler to the emission order.
            _prev_mm = [None]

            def mm(*args, **kw):
                inst = nc.tensor.matmul(*args, **kw)
                raw = getattr(inst, "ins", inst)
                if _prev_mm[0] is not None:
                    add_dep_helper(raw, _prev_mm[0], sync=False,
                                   reason="pe-stream-order")
                _prev_mm[0] = raw
                return inst

            nxt = None              # prefetched (noise_sb, xta_sb) for chunk+1
            carry_pvs = {}          # cross-chunk prefilled psum tiles
            carry_y = None          # previous chunk's deferred last y-block
            prev_TC = None
            for rep in range(reps):
                prev_hist = None
                for ck, (ts0, TCk) in enumerate(CHUNKS):
                    NBLK = TCk // SB
                    BLOCKS = [(b * SB, SB) for b in range(NBLK)]
                    if nxt is None:
                        # cold start: block 0's inputs ride the const DMAs,
                        # so only steps SB.. wait on these chunk DMAs
                        noise_sb = noisep.tile([128, TCk, 2 * BC], F8)
                        xta_sb = xtap.tile([4, TCk, BC], F16)
                        nc.sync.dma_start(out=xta_sb[:],
                                          in_=xta_d[:, ts0:ts0 + TCk, :])
                        nc.sync.dma_start(out=noise_sb[:],
                                          in_=noise_d[:, ts0:ts0 + TCk, :])
                    else:
                        noise_sb, xta_sb = nxt
                    # prefetch next chunk's inputs ahead of this chunk's
                    # y-output DMA in SP program order (SP issues in order and
                    # blocks on each DMA's waits)
                    if ck + 1 < len(CHUNKS) or rep + 1 < reps:
                        nts0, nTC = CHUNKS[(ck + 1) % len(CHUNKS)]
                        n2 = noisep.tile([128, nTC, 2 * BC], F8, name="noise2")
                        nc.sync.dma_start(out=n2[:],
                                          in_=noise_d[:, nts0:nts0 + nTC, :])
                        x2 = xtap.tile([4, nTC, BC], F16, name="xta2")
                        nc.sync.dma_start(out=x2[:],
                                          in_=xta_d[:, nts0:nts0 + nTC, :])
                        nxt = (n2, x2)
                    else:
                        nxt = None
                    noise_r = noise_sb[:].rearrange("p t (c b) -> p t c b", c=2)
                    noise_r2 = (nxt[0][:].rearrange("p t (c b) -> p t c b", c=2)
                                if nxt is not None else None)
                    # hist slot s holds state r_{ts0+s}; slot 0 = carry-in
                    hist = histp.tile([128, TCk + 1, 2, BC], F16)
                    ysb = ysbp.tile([3, TCk, BC], F16)
                    if ck == 0:
                        nc.vector.memset(hist[:, 0], 0.0)

                    pvs = carry_pvs
                    carry_pvs = {}

                    def emit_drive(key, blk, g, m_c, xt):
                        # PSUM bank protocol: exactly one start=True per bank
                        # (pending-zero is marked at 2KB-bank granularity), so
                        # only each group-tile's FIRST matmul starts; tiles
                        # are <= one bank and bank-aligned by pad_slot_size
                        b0, bn = BLOCKS[blk]
                        gsl = slice(g * GB, (g + 1) * GB)
                        if (key, g) not in pvs:
                            pvs[(key, g)] = pvp.tile([128, bn, 2, GB], F32,
                                                     name="pv", tag="pv")
                        mm(pvs[(key, g)][:, :, m_c, :],
                           win_sb[0:4, m_c * 128:(m_c + 1) * 128],
                           xt[:, b0:b0 + bn, gsl],
                           start=(m_c == 0), stop=False, skip_group_check=True)

                    def emit_noise(key, blk, g, s0, s1, nr):
                        b0, bn = BLOCKS[blk]
                        gsl = slice(g * GB, (g + 1) * GB)
                        for s in range(s0, min(s1, bn)):
                            mm(pvs[(key, g)][:, s], id_sb[:],
                               nr[:, b0 + s, :, gsl],
                               start=False, stop=False, skip_group_check=True)

                    def emit_y_at(key, b0, bn, hist_, ysb_, part):
                        # part 0: k-chunk-0 matmul; part 1: k-chunk-1 + copy
                        if part == 0:
                            pvs[key] = pyp.tile([3, bn, BC], F32,
                                                name="py", tag="py")
                        py = pvs[key]
                        mm(py[:],
                           wout_sb[:, part * 3:(part + 1) * 3],
                           hist_[:, 1 + b0:1 + b0 + bn, part, :],
                           start=(part == 0), stop=(part == 1),
                           skip_group_check=True)
                        if part == 1:
                            nc.scalar.copy(ysb_[:, b0:b0 + bn, :], py[:])
                            del pvs[key]

                    def emit_y(blk, part):
                        b0, bn = BLOCKS[blk]
                        emit_y_at(("y", blk), b0, bn, hist, ysb, part)

                    for blk in range(NBLK):
                        b0, bn = BLOCKS[blk]
                        if blk == 0 and (0, 0) not in pvs:
                            # very first chunk (or no prefetch): prefill from
                            # the const-packed block-0 copies (already local)
                            for g in range(G):
                                emit_drive(0, 0, g, 0, xta0_sb)
                                emit_drive(0, 0, g, 1, xta0_sb)
                            for g in range(G):
                                emit_noise(0, 0, g, 0, SB, noise0_sb)
                        # next emission target: block blk+1, or the NEXT
                        # chunk's block 0 (using the prefetched tiles)
                        if blk + 1 < NBLK:
                            nkey, nblk, nxta, nnr = blk + 1, blk + 1, xta_sb, noise_r
                        elif noise_r2 is not None:
                            nkey, nblk, nxta, nnr = "n0", 0, nxt[1], noise_r2
                        else:
                            nkey = None
                        for s in range(bn):
                            l = b0 + s
                            # filler emissions: issued while this step's w4
                            # matmuls wait on the previous state update
                            if s == 1 and blk > 0:
                                emit_y(blk - 1, 0)
                            elif s == 2 and blk > 0:
                                emit_y(blk - 1, 1)
                            elif s == 1 and blk == 0 and carry_y is not None:
                                emit_y_at(("ycarry",), *carry_y[:4], 0)
                            elif s == 2 and blk == 0 and carry_y is not None:
                                emit_y_at(("ycarry",), *carry_y[:4], 1)
                                pt0, pTC = carry_y[4:]
                                nc.sync.dma_start(
                                    out=y_d[:, pt0:pt0 + pTC, :],
                                    in_=carry_y[3][:])
                                carry_y = None
                            elif s == 3 and nkey is not None:
                                for g in range(G):
                                    emit_drive(nkey, nblk, g, 0, nxta)
                            elif s == 4 and nkey is not None:
                                for g in range(G):
                                    emit_drive(nkey, nblk, g, 1, nxta)
                            elif s == 5 and nkey is not None:
                                emit_noise(nkey, nblk, 0, 0, SB, nnr)
                            elif s == 6 and nkey is not None and G > 1:
                                emit_noise(nkey, nblk, 1, 0, SB, nnr)
                            if l == 0 and ck > 0:
                                rd, rs = prev_hist, prev_TC
                            else:
                                rd, rs = hist, l
                            for g in range(G):
                                gsl = slice(g * GB, (g + 1) * GB)
                                pv = pvs[(blk, g)]
                                for k_c in range(2):
                                    for m_c in range(2):
                                        mm(pv[:, s, m_c],
                                           w4_sb[:, (2 * k_c + m_c) * 128:
                                                 (2 * k_c + m_c + 1) * 128],
                                           rd[:, rs, k_c, gsl],
                                           start=False, stop=(k_c == 1),
                                           skip_group_check=True)
                                # H' = max((1-a)*H, S1)  (single fused DVE op)
                                nc.vector.scalar_tensor_tensor(
                                    out=hist[:, l + 1, :, gsl],
                                    in0=rd[:, rs, :, gsl],
                                    scalar=DECAY,
                                    in1=pv[:, s],
                                    op0=mybir.AluOpType.mult,
                                    op1=mybir.AluOpType.max)
                        if blk > 0:
                            for g in range(G):
                                del pvs[(blk - 1, g)]
                    if nxt is not None:
                        # defer this chunk's last y-block AND its output DMA
                        # into the next chunk's filler slots (off the chain)
                        carry_y = (BLOCKS[NBLK - 1][0], SB, hist, ysb,
                                   ts0, TCk)
                    else:
                        # final chunk: ship all but the last block's output
                        # early, leaving only a tiny DMA after the last y
                        lb0 = BLOCKS[NBLK - 1][0]
                        nc.sync.dma_start(out=y_d[:, ts0:ts0 + lb0, :],
                                          in_=ysb[:, 0:lb0])
                        emit_y(NBLK - 1, 0)
                        emit_y(NBLK - 1, 1)
                        nc.sync.dma_start(out=y_d[:, ts0 + lb0:ts0 + TCk, :],
                                          in_=ysb[:, lb0:])
                    for g in range(G):
                        if ("n0", g) in pvs:
                            carry_pvs[(0, g)] = pvs.pop(("n0", g))
                    prev_hist, prev_TC = hist, TCk
    nc.finalize()
    return nc


def get_nc(T=1000, TC=96, SB=8, G=2, reps=1):
    key = (T, TC, SB, G, reps)
    if key not in _CACHE:
        _CACHE[key] = _build(T, TC, SB, G, reps)
    return _CACHE[key]


def make_inputs(x, noise, W_in, W_rec, W_out_w, W_out_b, bias):
    """Host-side shard + layout prep.  Returns in_maps for 8 cores.

    Exponential rescaling: the device recurrence uses decay d = fp16(0.8),
    slightly below the true 0.8.  Because relu is positively homogeneous,
    running the recurrence on r~_t = c^t r_t with c = d/0.8 (so 0.8*c = d
    exactly), drive scaled by c^(t+1), and the output rescaled by c^-(t+1)
    on the host reproduces the true-decay dynamics exactly.
    """
    x = np.asarray(x, np.float32)
    noise = np.asarray(noise, np.float32)
    W_in = np.asarray(W_in, np.float32)
    W_rec = np.asarray(W_rec, np.float32)
    W_out_w = np.asarray(W_out_w, np.float32)
    W_out_b = np.asarray(W_out_b, np.float32)
    bias = np.asarray(bias, np.float32)
    B, T, _ = x.shape

    cfac = DECAY / (1.0 - ALPHA)                       # 0.99975586
    tfac = np.power(cfac, np.arange(1, T + 1), dtype=np.float64).astype(np.float32)

    # W4 chunks carry the state decay on their diagonal: W_rec's diagonal is
    # zero, so chunk (k,k)'s diagonal becomes fp16(1-alpha) exactly.
    cpack = np.zeros((128, 774 + 8 * BC), np.float16)  # w4|win|wout|xta blk0
    wrt = ALPHA * cfac * W_rec.T + DECAY * np.eye(256, dtype=np.float32)
    wrt = wrt.astype(np.float16)                       # [k, m]
    for k_c in range(2):
        for m_c in range(2):
            cpack[:, (2 * k_c + m_c) * 128:(2 * k_c + m_c + 1) * 128] = \
                wrt[128 * k_c:128 * (k_c + 1), 128 * m_c:128 * (m_c + 1)]
    ident = np.zeros((128, 128 + 8 * 2 * BC), F8NP)    # I/16 | noise blk0
    ident[:, 0:128] = (np.eye(128, dtype=np.float32) / NOISE_PREMUL).astype(F8NP)
    cpack[:3, 512:768] = (ALPHA * W_in.T).astype(np.float16)
    cpack[3, 512:768] = (ALPHA * bias).astype(np.float16)
    wt = W_out_w.T.astype(np.float16)              # [n, 3]
    for k_c in range(2):
        cpack[:, 768 + 3 * k_c:768 + 3 * (k_c + 1)] = \
            wt[128 * k_c:128 * (k_c + 1)]

    nscale = ALPHA * NOISE_SCALE
    in_maps = []
    for c in range(NCORES):
        b0 = c * BC
        nz = (noise[b0:b0 + BC] * (NOISE_PREMUL * nscale * tfac[None, :, None])
              ).astype(F8NP)                           # [32, T, 256]
        nzt = np.ascontiguousarray(
            nz.reshape(BC, T, 2, 128).transpose(3, 1, 2, 0)).reshape(128, T, 2 * BC)
        xc = x[b0:b0 + BC] * tfac[None, :, None]       # [32, T, 3]
        xta = np.empty((4, T, BC), np.float16)
        xta[:3] = xc.transpose(2, 1, 0).astype(np.float16)
        xta[3] = tfac[:, None]
        cpk = cpack.copy()
        cpk[0:4, 774:774 + 8 * BC] = xta[:, 0:8, :].reshape(4, 8 * BC)
        idp = ident.copy()
        idp[:, 128:128 + 8 * 2 * BC] = nzt[:, 0:8, :].reshape(128, 8 * 2 * BC)
        in_maps.append({
            "noiset": nzt, "xta": xta, "cpack": cpk, "ident": idp,
        })
    return in_maps


def gather_output(results, B, T, W_out_b):
    cfac = DECAY / (1.0 - ALPHA)
    inv = np.power(cfac, -np.arange(1, T + 1), dtype=np.float64).astype(np.float32)
    out = np.empty((B, T, 3), np.float32)
    for c in range(NCORES):
        out[c * BC:(c + 1) * BC] = results[c]["y"].transpose(2, 1, 0).astype(np.float32)
    out *= inv[None, :, None]
    out += np.asarray(W_out_b, np.float32)[None, None, :]
    return out


def kernel(x, noise, W_in, W_rec, W_out_w, W_out_b, bias):
    x = np.asarray(x, np.float32)
    B, T, _ = x.shape
    nc = get_nc(T=T)
    in_maps = make_inputs(x, noise, W_in, W_rec, W_out_w, W_out_b, bias)
    res = run_bass_kernel_spmd(nc, in_maps, list(range(NCORES)))
    return gather_output(res.results, B, T, W_out_b)
